# revision 28
# baseline (speedup 1.0000x reference)
"""MoE router gate (DeepSeek-V3 style) on 8 Trainium2 NeuronCores.

Math (per token):
  logits = x @ w.T            [N=16384, E=256], D=7168, fp32
  scores = sigmoid(logits)
  s      = scores + bias
  group top-2 sums over 8 groups of 32 -> keep top-4 groups
  indices = top-8 of s within kept groups
  weights = renormalize(scores[indices]) * 2.5

Sharding: data-parallel over tokens (2048/core); w+bias replicated.

GEMM strategy (per 128-deep contraction chunk, experts-stationary):
  hi pass : (x*2^12 fp16) @ (wh*2^13 fp16)              -> 2^25 * xh@wh
  corr    : one fp8 DoubleRow matmul contracts BOTH corrections:
            plane0 (w-quant): wl8=fp8((w-wh)*2^20) x xh8=fp8(x*2^5)
            plane1 (x-quant): wh8=fp8(wh*2^9)      x xl8=fp8((x-xh)*2^16)
            both products land at 2^25 scale -> shared PSUM accumulation.
  scores = sigmoid(psum * 2^-25) on the scalar engine (free combine).
Output tiles are [expert, token]; a PE transpose (fp32, via identity)
flips each 128-token block to [token, expert] for the DVE routing ops.
xh8 is cast on-device from the fp16 x tile (saves 14.7MB/core of DMA);
cast jobs alternate between the DVE and scalar engines.
"""

import sys
import threading

sys.path.insert(0, "/opt/trn_rl_repo")

import numpy as np
import ml_dtypes

import concourse.bass as bass
import concourse.bacc as bacc
import concourse.mybir as mybir
import concourse.tile as tile
from concourse.bass_utils import run_bass_kernel_spmd
from concourse.masks import make_identity

N_TOK = 16384
D = 7168
E = 256
N_CORES = 8
NSH = N_TOK // N_CORES          # tokens per core
N_CHUNK = D // 128              # 56 contraction chunks
N_GRP = 7                       # w/x DMA groups
GCH = N_CHUNK // N_GRP          # 8 chunks per group
OCT = 256                       # tokens per octant (matmul moving width)
N_OCT = NSH // OCT              # 8
N_GROUPS = 8
GSIZE = E // N_GROUPS           # 32
TOPK = 8
ROUTE_SCALE = 2.5
NEG_BIG = 1.0e30
PS_SCALE = 2.0 ** -25           # sigmoid scale undoing operand pre-scaling

_cached = {}


def _build_nc():
    fp16 = mybir.dt.float16
    fp8 = mybir.dt.float8e4
    f32 = mybir.dt.float32
    u32 = mybir.dt.uint32

    nc = bacc.Bacc(trn_type="TRN2", target_bir_lowering=False)

    xh_d = nc.dram_tensor("xh", [128, N_OCT, N_CHUNK, OCT], fp16, kind="ExternalInput")
    xl8_d = nc.dram_tensor("xl8", [128, N_OCT, N_CHUNK, OCT], fp8, kind="ExternalInput")
    # octant 0's xh8 plane comes pre-cast from the host: no cast op sits on
    # the critical fill path (casts for o>=1 run in steady-state slack)
    xh8o0_d = nc.dram_tensor("xh8o0", [128, N_CHUNK, OCT], fp8, kind="ExternalInput")
    w_d = nc.dram_tensor("w", [128, N_CHUNK, E], fp16, kind="ExternalInput")
    w8_d = nc.dram_tensor("w8", [128, N_CHUNK, 2, E], fp8, kind="ExternalInput")
    bias_d = nc.dram_tensor("bias", [128, E], f32, kind="ExternalInput")
    wts_d = nc.dram_tensor("wts", [NSH, TOPK], f32, kind="ExternalOutput")
    idx_d = nc.dram_tensor("idx", [NSH, TOPK], mybir.dt.int32, kind="ExternalOutput")

    with tile.TileContext(nc) as tc:
        with (
            tc.tile_pool(name="wpool", bufs=1) as wpool,
            tc.tile_pool(name="xpool", bufs=2) as xpool,
            tc.tile_pool(name="spool", bufs=2) as spool,
            tc.tile_pool(name="tiny", bufs=2) as tiny,
            tc.tile_pool(name="psacc", bufs=2, space="PSUM") as psacc,
            tc.tile_pool(name="pstr", bufs=2, space="PSUM") as pstr,
        ):
            # --- resident weights, loaded per group; first x octant interleaved
            w_sb, w8_sb = [], []
            xh_t = [[None] * N_OCT for _ in range(N_GRP)]
            x8_t = [[None] * N_OCT for _ in range(N_GRP)]

            def load_x(g, o, split_cast=False):
                r0, r1 = g * GCH, (g + 1) * GCH
                xt = xpool.tile([128, GCH, OCT], fp16, tag=f"xh{g}", bufs=2)
                nc.sync.dma_start(xt[:, :, :], xh_d[:, o, r0:r1, :])
                x8 = xpool.tile([128, 2, GCH, OCT], fp8, tag=f"x8{g}", bufs=2)
                # plane-major: both the xl8 DMA dst and the cast dst are
                # contiguous 2KB runs; gpsimd queue keeps desc-gen off sync
                nc.gpsimd.dma_start(x8[:, 1, :, :], xl8_d[:, o, r0:r1, :])
                # plane 0: xh8 = fp8(x*2^5) = fp8(xh_scaled * 2^-7), cast on device
                if o == 0:
                    nc.gpsimd.dma_start(x8[:, 0, :, :], xh8o0_d[:, r0:r1, :])
                elif (g + o) % 2 == 0:
                    nc.vector.tensor_scalar(
                        x8[:, 0, :, :], xt[:, :, :], 2.0 ** -7, None,
                        op0=mybir.AluOpType.mult,
                    )
                else:
                    nc.scalar.activation(
                        x8[:, 0, :, :], xt[:, :, :],
                        mybir.ActivationFunctionType.Copy, scale=2.0 ** -7,
                    )
                xh_t[g][o] = xt
                x8_t[g][o] = x8

            def load_w(g):
                r0, r1 = g * GCH, (g + 1) * GCH
                wg = wpool.tile([128, GCH, E], fp16, tag=f"w{g}")
                nc.sync.dma_start(wg[:, :, :], w_d[:, r0:r1, :])
                w_sb.append(wg)
                w8g = wpool.tile([128, GCH, 2, E], fp8, tag=f"w8{g}")
                nc.sync.dma_start(w8g[:, :, :, :], w8_d[:, r0:r1, :, :])
                w8_sb.append(w8g)

            # group 0's x before w so the first matmul's operands land first
            load_x(0, 0)
            load_w(0)
            bias_sb = wpool.tile([128, E], f32, tag="bias")
            nc.scalar.dma_start(bias_sb[:, :], bias_d[:, :])
            for g in range(1, N_GRP):
                load_x(g, 0)
                load_w(g)
            ident = wpool.tile([128, 128], f32, tag="ident")
            make_identity(nc, ident[:, :])

            for o in range(N_OCT):
                acc0 = psacc.tile([128, OCT], f32, tag="acc0")
                acc1 = psacc.tile([128, OCT], f32, tag="acc1")
                acc = [acc0, acc1]
                # lag-2 group schedule: hi(g) blocks run two groups ahead of
                # DR(g) blocks, so each group's xl8 DMA + fp8 cast completes
                # under ~7us of hi work; blocks of 16 keep fp16<->fp8 mode
                # transitions rare (~15ns each on the first fp8 matmul)
                def hi_block(g):
                    for ci in range(GCH):
                        for eh in (0, 1):
                            esl = slice(eh * 128, (eh + 1) * 128)
                            nc.tensor.matmul(
                                acc[eh][:, :],
                                w_sb[g][:, ci, esl],
                                xh_t[g][o][:, ci, :],
                                start=(g == 0 and ci == 0),
                                stop=False,
                            )

                def dr_block(g):
                    for ci in range(GCH):
                        last = g == N_GRP - 1 and ci == GCH - 1
                        for eh in (0, 1):
                            esl = slice(eh * 128, (eh + 1) * 128)
                            nc.tensor.matmul(
                                acc[eh][:, :],
                                w8_sb[g][:, ci, :, esl],
                                x8_t[g][o][:, :, ci, :],
                                start=False, stop=last,
                                perf_mode=mybir.MatmulPerfMode.DoubleRow,
                            )

                hi_block(0)
                hi_block(1)
                for g in range(2, N_GRP):
                    hi_block(g)
                    dr_block(g - 2)
                dr_block(N_GRP - 2)
                dr_block(N_GRP - 1)
                if o + 1 < N_OCT:
                    for g in range(N_GRP):
                        load_x(g, o + 1)

                # scores in [expert, token] orientation
                sc_et0 = spool.tile([128, OCT], f32, tag="sc_et0")
                sc_et1 = spool.tile([128, OCT], f32, tag="sc_et1")
                sc_et = [sc_et0, sc_et1]
                for eh in (0, 1):
                    nc.scalar.activation(
                        sc_et[eh][:, :], acc[eh][:, :],
                        mybir.ActivationFunctionType.Sigmoid, scale=PS_SCALE,
                    )

                for j in range(OCT // 128):
                    ts0 = o * OCT + j * 128
                    jsl = slice(j * 128, (j + 1) * 128)
                    psT = pstr.tile([128, E], f32, tag="psT")
                    for eh in (0, 1):
                        nc.tensor.transpose(
                            psT[:, eh * 128:(eh + 1) * 128],
                            sc_et[eh][:, jsl], ident[:, :],
                        )

                    scores = spool.tile([128, E], f32, tag="scores")
                    nc.scalar.copy(scores[:, :], psT[:, :])
                    s = spool.tile([128, E], f32, tag="s")
                    nc.vector.tensor_add(s[:, :], psT[:, :], bias_sb[:, :])

                    gtop = tiny.tile([128, N_GROUPS, 8], f32, tag="gtop")
                    for gi in range(N_GROUPS):
                        nc.vector.max(gtop[:, gi, :], s[:, gi * GSIZE:(gi + 1) * GSIZE])
                    gs = tiny.tile([128, N_GROUPS], f32, tag="gs")
                    nc.vector.tensor_add(gs[:, :], gtop[:, :, 0], gtop[:, :, 1])
                    gsort = tiny.tile([128, 8], f32, tag="gsort")
                    nc.vector.max(gsort[:, :], gs[:, :])
                    # amask = (gs < gsort[3]) * -BIG   {0 keep, -BIG drop}
                    amask = tiny.tile([128, N_GROUPS], f32, tag="amask")
                    nc.vector.tensor_scalar(
                        amask[:, :], gs[:, :], gsort[:, 3:4], -NEG_BIG,
                        op0=mybir.AluOpType.is_lt, op1=mybir.AluOpType.mult,
                    )

                    # smask = s + amask broadcast over each group of 32
                    smask = spool.tile([128, N_GROUPS, GSIZE], f32, tag="smask")
                    am_ap = amask[:, :]
                    am_bc = bass.AP(am_ap.tensor, am_ap.offset,
                                    [am_ap.ap[0], am_ap.ap[1], (0, GSIZE)])
                    nc.vector.tensor_tensor(
                        smask[:, :, :],
                        s[:, :].rearrange("p (g e) -> p g e", g=N_GROUPS),
                        am_bc, op=mybir.AluOpType.add,
                    )

                    smask2 = smask[:, :, :].rearrange("p g e -> p (g e)")
                    top8v = tiny.tile([128, TOPK], f32, tag="top8v")
                    nc.vector.max(top8v[:, :], smask2)
                    top8i = tiny.tile([128, TOPK], u32, tag="top8i")
                    nc.vector.max_index(top8i[:, :], top8v[:, :], smask2)

                    wsel = tiny.tile([128, TOPK], f32, tag="wsel")
                    scratch = spool.tile([128, E], f32, tag="scratch")
                    for k in range(TOPK):
                        nc.vector.scalar_tensor_tensor(
                            scratch[:, :], smask2, top8v[:, k:k + 1], scores[:, :],
                            op0=mybir.AluOpType.is_equal, op1=mybir.AluOpType.mult,
                            accum_out=wsel[:, k:k + 1],
                        )

                    ssum = tiny.tile([128, 1], f32, tag="ssum")
                    nc.vector.reduce_sum(ssum[:, :], wsel[:, :], axis=mybir.AxisListType.X)
                    rec = tiny.tile([128, 1], f32, tag="rec")
                    nc.vector.reciprocal(rec[:, :], ssum[:, :])
                    wout = tiny.tile([128, TOPK], f32, tag="wout")
                    nc.vector.tensor_scalar(
                        wout[:, :], wsel[:, :], rec[:, 0:1], ROUTE_SCALE,
                        op0=mybir.AluOpType.mult, op1=mybir.AluOpType.mult,
                    )

                    nc.sync.dma_start(wts_d[ts0:ts0 + 128, :], wout[:, :])
                    nc.sync.dma_start(
                        idx_d[ts0:ts0 + 128, :], top8i[:, :].bitcast(mybir.dt.int32)
                    )
    nc.finalize()
    return nc


def _host_prep(x, weight, bias):
    """Pre-scale/split to fp16+fp8 operands, partition-major layouts."""
    x = np.asarray(x, dtype=np.float32)
    weight = np.asarray(weight, dtype=np.float32)
    bias = np.asarray(bias, dtype=np.float32)
    e4 = ml_dtypes.float8_e4m3

    wh16 = weight.astype(np.float16)                     # [E, D]
    wh32 = wh16.astype(np.float32)
    # [E, D] -> [128, 56, E]
    def wlay(a):
        return np.ascontiguousarray(a.T.reshape(N_CHUNK, 128, E).transpose(1, 0, 2))

    w_hbm = wlay(wh16 * np.float16(2.0 ** 13))
    wl8 = wlay(np.clip((weight - wh32) * 2.0 ** 20, -240, 240).astype(e4))
    wh8 = wlay((wh32 * 2.0 ** 9).astype(e4))
    w8_hbm = np.ascontiguousarray(np.stack([wl8, wh8], axis=2))  # [128, 56, 2, E]
    bias_rep = np.ascontiguousarray(np.broadcast_to(bias[None, :], (128, E)))

    in_maps = [None] * N_CORES

    def prep_core(c):
        xs = x[c * NSH:(c + 1) * NSH, :]                 # [2048, 7168]
        xh16 = xs.astype(np.float16)
        xh32 = xh16.astype(np.float32)
        # [N, D] -> [128, N_OCT, 56, OCT]: x[n, ck*128+p] -> [p, n//OCT, ck, n%OCT]
        def xlay(a):
            return np.ascontiguousarray(
                a.T.reshape(N_CHUNK, 128, N_OCT, OCT).transpose(1, 2, 0, 3)
            )

        xh_hbm = xlay(xh16 * np.float16(2.0 ** 12))
        xl8_hbm = xlay(np.clip((xs - xh32) * 2.0 ** 16, -240, 240).astype(e4))
        xh8o0 = np.ascontiguousarray(
            (xs[0:OCT] * 2.0 ** 5).astype(e4).T.reshape(N_CHUNK, 128, OCT)
            .transpose(1, 0, 2)
        )
        in_maps[c] = {
            "xh": xh_hbm,
            "xl8": xl8_hbm,
            "xh8o0": xh8o0,
            "w": w_hbm,
            "w8": w8_hbm,
            "bias": bias_rep,
        }

    threads = [threading.Thread(target=prep_core, args=(c,)) for c in range(N_CORES)]
    for th in threads:
        th.start()
    for th in threads:
        th.join()
    return in_maps


def kernel(x, weight, bias, _trace=False):
    if "nc" not in _cached:
        _cached["nc"] = _build_nc()
    nc = _cached["nc"]
    in_maps = _host_prep(x, weight, bias)
    res = run_bass_kernel_spmd(
        nc, in_maps, core_ids=list(range(N_CORES)), trace=_trace
    )
    _cached["last_result"] = res
    wts = np.concatenate([r["wts"] for r in res.results], axis=0)
    idx = np.concatenate([r["idx"] for r in res.results], axis=0)
    return wts, idx


# revision 30
# speedup vs baseline: 1.0220x; 1.0220x over previous
"""MoE router gate (DeepSeek-V3 style) on 8 Trainium2 NeuronCores.

Math (per token):
  logits = x @ w.T            [N=16384, E=256], D=7168, fp32
  scores = sigmoid(logits)
  s      = scores + bias
  group top-2 sums over 8 groups of 32 -> keep top-4 groups
  indices = top-8 of s within kept groups
  weights = renormalize(scores[indices]) * 2.5

Sharding: data-parallel over tokens (2048/core); w+bias replicated.

GEMM strategy (per 128-deep contraction chunk, experts-stationary):
  hi pass : (x*2^12 fp16) @ (wh*2^13 fp16)              -> 2^25 * xh@wh
  corr    : one fp8 DoubleRow matmul contracts BOTH corrections:
            plane0 (w-quant): wl8=fp8((w-wh)*2^20) x xh8=fp8(x*2^5)
            plane1 (x-quant): wh8=fp8(wh*2^9)      x xl8=fp8((x-xh)*2^16)
            both products land at 2^25 scale -> shared PSUM accumulation.
  scores = sigmoid(psum * 2^-25) on the scalar engine (free combine).
Output tiles are [expert, token]; a PE transpose (fp32, via identity)
flips each 128-token block to [token, expert] for the DVE routing ops.
xh8 is cast on-device from the fp16 x tile (saves 14.7MB/core of DMA);
cast jobs alternate between the DVE and scalar engines.
"""

import sys
import threading

sys.path.insert(0, "/opt/trn_rl_repo")

import numpy as np
import ml_dtypes

import concourse.bass as bass
import concourse.bacc as bacc
import concourse.mybir as mybir
import concourse.tile as tile
from concourse.bass_utils import run_bass_kernel_spmd
from concourse.masks import make_identity

N_TOK = 16384
D = 7168
E = 256
N_CORES = 8
NSH = N_TOK // N_CORES          # tokens per core
N_CHUNK = D // 128              # 56 contraction chunks
N_GRP = 7                       # w/x DMA groups
GCH = N_CHUNK // N_GRP          # 8 chunks per group
OCT = 256                       # tokens per octant (matmul moving width)
N_OCT = NSH // OCT              # 8
N_GROUPS = 8
GSIZE = E // N_GROUPS           # 32
TOPK = 8
ROUTE_SCALE = 2.5
NEG_BIG = 1.0e30
PS_SCALE = 2.0 ** -25           # sigmoid scale undoing operand pre-scaling

_cached = {}


def _build_nc():
    fp16 = mybir.dt.float16
    fp8 = mybir.dt.float8e4
    f32 = mybir.dt.float32
    u32 = mybir.dt.uint32

    nc = bacc.Bacc(trn_type="TRN2", target_bir_lowering=False)

    xh_d = nc.dram_tensor("xh", [128, N_OCT, N_CHUNK, OCT], fp16, kind="ExternalInput")
    xl8_d = nc.dram_tensor("xl8", [128, N_OCT, N_CHUNK, OCT], fp8, kind="ExternalInput")
    w_d = nc.dram_tensor("w", [128, N_CHUNK, E], fp16, kind="ExternalInput")
    w8_d = nc.dram_tensor("w8", [128, N_CHUNK, 2, E], fp8, kind="ExternalInput")
    bias_d = nc.dram_tensor("bias", [128, E], f32, kind="ExternalInput")
    wts_d = nc.dram_tensor("wts", [NSH, TOPK], f32, kind="ExternalOutput")
    idx_d = nc.dram_tensor("idx", [NSH, TOPK], mybir.dt.int32, kind="ExternalOutput")

    with tile.TileContext(nc) as tc:
        with (
            tc.tile_pool(name="wpool", bufs=1) as wpool,
            tc.tile_pool(name="xpool", bufs=2) as xpool,
            tc.tile_pool(name="spool", bufs=2) as spool,
            tc.tile_pool(name="tiny", bufs=2) as tiny,
            tc.tile_pool(name="psacc", bufs=2, space="PSUM") as psacc,
            tc.tile_pool(name="pstr", bufs=2, space="PSUM") as pstr,
        ):
            # --- resident weights, loaded per group; first x octant interleaved
            w_sb, w8_sb = [], []
            xh_t = [[None] * N_OCT for _ in range(N_GRP)]
            x8_t = [[None] * N_OCT for _ in range(N_GRP)]

            def load_x(g, o, split_cast=False):
                r0, r1 = g * GCH, (g + 1) * GCH
                xt = xpool.tile([128, GCH, OCT], fp16, tag=f"xh{g}", bufs=2)
                nc.sync.dma_start(xt[:, :, :], xh_d[:, o, r0:r1, :])
                x8 = xpool.tile([128, 2, GCH, OCT], fp8, tag=f"x8{g}", bufs=2)
                # plane-major: both the xl8 DMA dst and the cast dst are
                # contiguous 2KB runs; gpsimd queue keeps desc-gen off sync
                nc.gpsimd.dma_start(x8[:, 1, :, :], xl8_d[:, o, r0:r1, :])
                # plane 0: xh8 = fp8(x*2^5) = fp8(xh_scaled * 2^-7), cast on device
                if split_cast:
                    half = GCH // 2
                    nc.vector.tensor_scalar(
                        x8[:, 0, 0:half, :], xt[:, 0:half, :], 2.0 ** -7, None,
                        op0=mybir.AluOpType.mult,
                    )
                    nc.scalar.activation(
                        x8[:, 0, half:GCH, :], xt[:, half:GCH, :],
                        mybir.ActivationFunctionType.Copy, scale=2.0 ** -7,
                    )
                elif (g + o) % 2 == 0:
                    nc.vector.tensor_scalar(
                        x8[:, 0, :, :], xt[:, :, :], 2.0 ** -7, None,
                        op0=mybir.AluOpType.mult,
                    )
                else:
                    nc.scalar.activation(
                        x8[:, 0, :, :], xt[:, :, :],
                        mybir.ActivationFunctionType.Copy, scale=2.0 ** -7,
                    )
                xh_t[g][o] = xt
                x8_t[g][o] = x8

            def load_w(g):
                r0, r1 = g * GCH, (g + 1) * GCH
                wg = wpool.tile([128, GCH, E], fp16, tag=f"w{g}")
                nc.sync.dma_start(wg[:, :, :], w_d[:, r0:r1, :])
                w_sb.append(wg)
                # w8 rides the gpsimd queue with xl8: the hi blocks need only
                # w+xh (sync queue), and lag-2 gives the DR operands ~7us slack
                w8g = wpool.tile([128, GCH, 2, E], fp8, tag=f"w8{g}")
                nc.gpsimd.dma_start(w8g[:, :, :, :], w8_d[:, r0:r1, :, :])
                w8_sb.append(w8g)

            # queue split: sync carries only xh (+outputs), gpsimd carries
            # w/w8/xl8, scalar carries bias — group 0 lands fastest and the
            # first matmuls start early
            load_w(0)
            load_x(0, 0, split_cast=True)
            bias_sb = wpool.tile([128, E], f32, tag="bias")
            nc.scalar.dma_start(bias_sb[:, :], bias_d[:, :])
            for g in range(1, N_GRP):
                load_w(g)
                load_x(g, 0, split_cast=True)
            ident = wpool.tile([128, 128], f32, tag="ident")
            make_identity(nc, ident[:, :])

            for o in range(N_OCT):
                acc0 = psacc.tile([128, OCT], f32, tag="acc0")
                acc1 = psacc.tile([128, OCT], f32, tag="acc1")
                acc = [acc0, acc1]
                # lag-2 group schedule: hi(g) blocks run two groups ahead of
                # DR(g) blocks, so each group's xl8 DMA + fp8 cast completes
                # under ~7us of hi work; blocks of 16 keep fp16<->fp8 mode
                # transitions rare (~15ns each on the first fp8 matmul)
                def hi_block(g):
                    for ci in range(GCH):
                        for eh in (0, 1):
                            esl = slice(eh * 128, (eh + 1) * 128)
                            nc.tensor.matmul(
                                acc[eh][:, :],
                                w_sb[g][:, ci, esl],
                                xh_t[g][o][:, ci, :],
                                start=(g == 0 and ci == 0),
                                stop=False,
                            )

                def dr_block(g):
                    for ci in range(GCH):
                        last = g == N_GRP - 1 and ci == GCH - 1
                        for eh in (0, 1):
                            esl = slice(eh * 128, (eh + 1) * 128)
                            nc.tensor.matmul(
                                acc[eh][:, :],
                                w8_sb[g][:, ci, :, esl],
                                x8_t[g][o][:, :, ci, :],
                                start=False, stop=last,
                                perf_mode=mybir.MatmulPerfMode.DoubleRow,
                            )

                hi_block(0)
                hi_block(1)
                for g in range(2, N_GRP):
                    hi_block(g)
                    dr_block(g - 2)
                dr_block(N_GRP - 2)
                dr_block(N_GRP - 1)
                if o + 1 < N_OCT:
                    for g in range(N_GRP):
                        load_x(g, o + 1)

                # scores in [expert, token] orientation
                sc_et0 = spool.tile([128, OCT], f32, tag="sc_et0")
                sc_et1 = spool.tile([128, OCT], f32, tag="sc_et1")
                sc_et = [sc_et0, sc_et1]
                for eh in (0, 1):
                    nc.scalar.activation(
                        sc_et[eh][:, :], acc[eh][:, :],
                        mybir.ActivationFunctionType.Sigmoid, scale=PS_SCALE,
                    )

                for j in range(OCT // 128):
                    ts0 = o * OCT + j * 128
                    jsl = slice(j * 128, (j + 1) * 128)
                    psT = pstr.tile([128, E], f32, tag="psT")
                    for eh in (0, 1):
                        nc.tensor.transpose(
                            psT[:, eh * 128:(eh + 1) * 128],
                            sc_et[eh][:, jsl], ident[:, :],
                        )

                    scores = spool.tile([128, E], f32, tag="scores")
                    nc.scalar.copy(scores[:, :], psT[:, :])
                    s = spool.tile([128, E], f32, tag="s")
                    nc.vector.tensor_add(s[:, :], psT[:, :], bias_sb[:, :])

                    gtop = tiny.tile([128, N_GROUPS, 8], f32, tag="gtop")
                    for gi in range(N_GROUPS):
                        nc.vector.max(gtop[:, gi, :], s[:, gi * GSIZE:(gi + 1) * GSIZE])
                    gs = tiny.tile([128, N_GROUPS], f32, tag="gs")
                    nc.vector.tensor_add(gs[:, :], gtop[:, :, 0], gtop[:, :, 1])
                    gsort = tiny.tile([128, 8], f32, tag="gsort")
                    nc.vector.max(gsort[:, :], gs[:, :])
                    # amask = (gs < gsort[3]) * -BIG   {0 keep, -BIG drop}
                    amask = tiny.tile([128, N_GROUPS], f32, tag="amask")
                    nc.vector.tensor_scalar(
                        amask[:, :], gs[:, :], gsort[:, 3:4], -NEG_BIG,
                        op0=mybir.AluOpType.is_lt, op1=mybir.AluOpType.mult,
                    )

                    # smask = s + amask broadcast over each group of 32
                    smask = spool.tile([128, N_GROUPS, GSIZE], f32, tag="smask")
                    am_ap = amask[:, :]
                    am_bc = bass.AP(am_ap.tensor, am_ap.offset,
                                    [am_ap.ap[0], am_ap.ap[1], (0, GSIZE)])
                    nc.vector.tensor_tensor(
                        smask[:, :, :],
                        s[:, :].rearrange("p (g e) -> p g e", g=N_GROUPS),
                        am_bc, op=mybir.AluOpType.add,
                    )

                    smask2 = smask[:, :, :].rearrange("p g e -> p (g e)")
                    top8v = tiny.tile([128, TOPK], f32, tag="top8v")
                    nc.vector.max(top8v[:, :], smask2)
                    top8i = tiny.tile([128, TOPK], u32, tag="top8i")
                    nc.vector.max_index(top8i[:, :], top8v[:, :], smask2)

                    wsel = tiny.tile([128, TOPK], f32, tag="wsel")
                    scratch = spool.tile([128, E], f32, tag="scratch")
                    for k in range(TOPK):
                        nc.vector.scalar_tensor_tensor(
                            scratch[:, :], smask2, top8v[:, k:k + 1], scores[:, :],
                            op0=mybir.AluOpType.is_equal, op1=mybir.AluOpType.mult,
                            accum_out=wsel[:, k:k + 1],
                        )

                    ssum = tiny.tile([128, 1], f32, tag="ssum")
                    nc.vector.reduce_sum(ssum[:, :], wsel[:, :], axis=mybir.AxisListType.X)
                    rec = tiny.tile([128, 1], f32, tag="rec")
                    nc.vector.reciprocal(rec[:, :], ssum[:, :])
                    wout = tiny.tile([128, TOPK], f32, tag="wout")
                    nc.vector.tensor_scalar(
                        wout[:, :], wsel[:, :], rec[:, 0:1], ROUTE_SCALE,
                        op0=mybir.AluOpType.mult, op1=mybir.AluOpType.mult,
                    )

                    nc.sync.dma_start(wts_d[ts0:ts0 + 128, :], wout[:, :])
                    nc.sync.dma_start(
                        idx_d[ts0:ts0 + 128, :], top8i[:, :].bitcast(mybir.dt.int32)
                    )
    nc.finalize()
    return nc


def _host_prep(x, weight, bias):
    """Pre-scale/split to fp16+fp8 operands, partition-major layouts."""
    x = np.asarray(x, dtype=np.float32)
    weight = np.asarray(weight, dtype=np.float32)
    bias = np.asarray(bias, dtype=np.float32)
    e4 = ml_dtypes.float8_e4m3

    wh16 = weight.astype(np.float16)                     # [E, D]
    wh32 = wh16.astype(np.float32)
    # [E, D] -> [128, 56, E]
    def wlay(a):
        return np.ascontiguousarray(a.T.reshape(N_CHUNK, 128, E).transpose(1, 0, 2))

    w_hbm = wlay(wh16 * np.float16(2.0 ** 13))
    wl8 = wlay(np.clip((weight - wh32) * 2.0 ** 20, -240, 240).astype(e4))
    wh8 = wlay((wh32 * 2.0 ** 9).astype(e4))
    w8_hbm = np.ascontiguousarray(np.stack([wl8, wh8], axis=2))  # [128, 56, 2, E]
    bias_rep = np.ascontiguousarray(np.broadcast_to(bias[None, :], (128, E)))

    in_maps = [None] * N_CORES

    def prep_core(c):
        xs = x[c * NSH:(c + 1) * NSH, :]                 # [2048, 7168]
        xh16 = xs.astype(np.float16)
        xh32 = xh16.astype(np.float32)
        # [N, D] -> [128, N_OCT, 56, OCT]: x[n, ck*128+p] -> [p, n//OCT, ck, n%OCT]
        def xlay(a):
            return np.ascontiguousarray(
                a.T.reshape(N_CHUNK, 128, N_OCT, OCT).transpose(1, 2, 0, 3)
            )

        xh_hbm = xlay(xh16 * np.float16(2.0 ** 12))
        xl8_hbm = xlay(np.clip((xs - xh32) * 2.0 ** 16, -240, 240).astype(e4))
        in_maps[c] = {
            "xh": xh_hbm,
            "xl8": xl8_hbm,
            "w": w_hbm,
            "w8": w8_hbm,
            "bias": bias_rep,
        }

    threads = [threading.Thread(target=prep_core, args=(c,)) for c in range(N_CORES)]
    for th in threads:
        th.start()
    for th in threads:
        th.join()
    return in_maps


def kernel(x, weight, bias, _trace=False):
    if "nc" not in _cached:
        _cached["nc"] = _build_nc()
    nc = _cached["nc"]
    in_maps = _host_prep(x, weight, bias)
    res = run_bass_kernel_spmd(
        nc, in_maps, core_ids=list(range(N_CORES)), trace=_trace
    )
    _cached["last_result"] = res
    wts = np.concatenate([r["wts"] for r in res.results], axis=0)
    idx = np.concatenate([r["idx"] for r in res.results], axis=0)
    return wts, idx
